# revision 12
# baseline (speedup 1.0000x reference)
"""BumpX pooling kernel for Trainium2 (8 NeuronCores, data-parallel over batch).

Math (per batch b, row l, position i, a = aa[b,l,i], d = |j - i|):
    arg_d = (d^2 - a^2) / (6a + 9)
    m_d   = 1 - gg(arg_d)        (the bump mask; underflows for d >= 7)
    out_i = sum_d m_d (x[i-d] + x[i+d]) / sum_d m_d n_valid(i, d)

Approximations (rel tolerance 2e-2; this lands ~6e-3):
  - m(t) = exp(-exp(g(t))), g fitted by a degree-4 polynomial over
    t in [-0.08, 4.01] (the full arg range for d <= 6); d=7 dropped.
  - mask/tap stacks held in bf16 (2x DVE rate on packed tensor_tensor);
    reductions accumulate in fp32.

Per-element chain: arg (2 DVE ops; 1/(6a+9) comes from ACT's
Ln(6a+9)/Exp with the affine folded into scale/bias) -> monic Horner q
(3 fused DVE ops) -> E1 = Exp(c4*q + c0) (ACT) -> m = Exp(-E1) (ACT,
emitted as a d>=1 slab plus a d=0 slab with bias -ln2 so m_0 comes out
pre-halved). The symmetric-tap stack xs_d = x[i-d] + x[i+d] is ONE DVE
op via a +/-1-stride view pair (xs_0 = 2x, compensated by the halved
m_0), giving num = reduce(m*xs) and den = 2*reduce(m). Row-edge taps
are removed with masked products + fixups entirely on GpSimd.

The stacks are processed in two position halves software-pipelined
across DVE and ACT; each half's output columns are DMAed as soon as its
numerator/denominator finish (SP stores half 0, ACT half 1). No engine
waits for output-DMA completion: the NEFF epilogue's per-queue drain
waits already guarantee the stores land before completion is signaled.

Layout per core (core = batch): partition p = c*16 + l (c = chunk of 128
positions, l = row); stacks are (128, 128, 7) with d innermost so the
d-reduction is a contiguous X-reduce. Inputs arrive as two 2D DMAs:
bf16 [x-halo(140) | edge-masks(49)] per partition, and fp32 aa (128x128).
"""

import numpy as np

import concourse.bass as bass
import concourse.mybir as mybir
from concourse.bass_utils import run_bass_kernel_spmd

F32 = mybir.dt.float32
BF16 = mybir.dt.bfloat16
L, F = 16, 1024
NC_COUNT = 8
W = 6          # max diagonal distance kept
ND = W + 1     # stack depth (d = 0..6)
HALO = W
XW = 128       # positions per chunk
HW_ = XW // 2  # position half width
NCH = F // XW  # 8 chunks
XROW = XW + 2 * HALO           # 140
PITCH = XROW + ND * ND         # 189: [x-halo | EC masks]
LN2 = float(np.log(2.0))

# g(t) = ln(-ln(m(t))) degree-4 weighted fit over t in [-0.08, 4.01]
GC = (-0.029456496983506418, 0.18552920622532633, -0.3712109527981173,
      1.1947827839859845, -0.8925694191796499)
C4, C3, C2, C1, C0 = GC
U3, U2, U1 = C3 / C4, C2 / C4, C1 / C4


class _FastBass(bass.Bass):
    """Skip the constructor's all-engine barrier (~3us): we never read the
    framework's const APs (all ACT biases are explicit tiles)."""

    def all_engine_barrier(self, *, sem_only: bool = False):
        if not getattr(self, "_init_barrier_skipped", False):
            self._init_barrier_skipped = True
            return
        return super().all_engine_barrier(sem_only=sem_only)


def build_bass():
    nc = _FastBass("TRN2", debug=False)

    xe_d = nc.dram_tensor("xe", [128, PITCH], BF16, kind="ExternalInput").ap()
    aa_d = nc.dram_tensor("aa", [128, XW], F32, kind="ExternalInput").ap()
    out_d = nc.dram_tensor("out", [128, XW], F32, kind="ExternalOutput").ap()

    def sb(name, shape, dt=F32):
        return nc.alloc_sbuf_tensor(name, shape, dt).ap()

    XE = sb("XE", [128, PITCH], BF16)  # [x-halo(140) | EC(7,7)(49)]
    A = sb("A", [128, XW])
    DSQ = sb("DSQ", [128, ND])
    CB0 = sb("CB0", [128, 1])          # 0.0   (ACT bias)
    CB9 = sb("CB9", [128, 1])          # 9.0   (ACT bias: Ln(6a+9))
    CG0 = sb("CG0", [128, 1])          # C0    (ACT bias for E1)
    CL2 = sb("CL2", [128, 1])          # -ln2  (ACT bias: halved m_0)
    WRM = sb("WRM", [128, 1])
    lden = sb("lden", [128, XW])
    rden = sb("rden", [128, XW])
    asq = sb("asq", [128, XW])
    arg = sb("arg", [128, XW, ND], BF16)
    q = sb("q", [128, XW, ND], BF16)
    E1 = sb("E1", [128, XW, ND])
    m = sb("m", [128, XW, ND], BF16)
    xs = sb("xs", [128, XW, ND], BF16)
    mp = sb("mp", [128, XW, ND], BF16)
    S = sb("S", [128, XW])
    den = sb("den", [128, XW])
    lden2 = sb("lden2", [128, XW])
    rdn = sb("rdn", [128, XW])
    num = sb("num", [128, XW])
    O = sb("O", [128, XW])
    et = sb("et", [128, ND, ND], BF16)  # edge products ([0:32] / [96:128])
    ered = sb("ered", [128, ND])

    # EC view: XE[:, 140:189] seen as (128, 7, 7) [k, d]
    EC = bass.AP(tensor=XE.tensor, offset=XROW,
                 ap=[[PITCH, 128], [ND, ND], [1, ND]])
    # xs operand views: elem (p, i, d) -> XE[p, HALO + i -/+ d]
    xm_v = bass.AP(tensor=XE.tensor, offset=HALO,
                   ap=[[PITCH, 128], [1, XW], [-1, ND]])
    xp_v = bass.AP(tensor=XE.tensor, offset=HALO,
                   ap=[[PITCH, 128], [1, XW], [1, ND]])

    AL = mybir.AluOpType
    AF = mybir.ActivationFunctionType

    def h(t, k):
        """Column-half slice of a (128, XW, ...) AP."""
        return t[:, k * HW_:(k + 1) * HW_]

    class Eng:
        """Engine op wrapper with minimal-dependency waits (see baseline)."""

        def __init__(self, eng, sem):
            self.eng, self.sem, self.n = eng, sem, 0
            self.waited = {}

        def wait(self, sem, val):
            key = id(sem)
            if self.waited.get(key, -1) < val:
                self.eng.wait_ge(sem, val)
                self.waited[key] = val

        def op(self, make_inst, after=0, waits=()):
            for sem, val in waits:
                self.wait(sem, val)
            if after:
                self.wait(self.sem, after)
            inst = make_inst()
            inst.then_inc(self.sem, 1)
            self.n += 1
            assert self.n >= after
            return inst

    with (
        nc.Block(no_gpsimd_drain=True) as block,
        nc.semaphore("s_a") as s_a,
        nc.semaphore("s_x") as s_x,
        nc.semaphore("s_fin") as s_fin,
        nc.semaphore("s_v") as s_v,      # DVE chain
        nc.semaphore("s_t") as s_t,      # ACT chain
        nc.semaphore("s_g") as s_g,      # GPSIMD chain
    ):
        # chain-count milestones
        G_CB = 4
        G_DSQ = 11
        G_ET = (12, 13)
        G_DEN = (14, 15)
        V_POLY = (7, 12)
        V_ERED = (15, 20)
        V_OUT = (21, 24)
        T_RDEN = 3
        T_MM = (4 + 1, 7 + 1)   # m (d>=1) slab done, per half
        T_M = (4 + 2, 7 + 2)    # full m (incl halved d=0), per half
        T_RDN = (11, 13)

        @block.sync
        def _(sync: bass.BassEngine):
            sync.dma_start(out=XE, in_=xe_d).then_inc(s_x, 16)
            sync.wait_ge(s_v, V_OUT[0])
            sync.dma_start(out=out_d[:, 0:HW_],
                           in_=O[:, 0:HW_]).then_inc(s_fin, 16)

        @block.gpsimd
        def _(g: bass.BassEngine):
            e = Eng(g, s_g)
            e.op(lambda: g.memset(CB0, 0.0))
            e.op(lambda: g.memset(CB9, 9.0))
            e.op(lambda: g.memset(CG0, float(C0)))
            e.op(lambda: g.memset(CL2, -LN2))
            assert e.n == G_CB, e.n
            for d in range(ND):
                e.op(lambda d=d: g.memset(DSQ[:, d:d + 1], float(d * d)))
            assert e.n == G_DSQ, e.n
            # edge products + reductions (d >= 1 only)
            e.op(lambda: g.tensor_tensor(et[0:32, :, 1:ND],
                                         m[0:32, 0:ND, 1:ND],
                                         EC[0:32, :, 1:ND], op=AL.mult),
                 waits=((s_t, T_MM[0]), (s_x, 16)))
            assert e.n == G_ET[0], e.n
            e.op(lambda: g.tensor_tensor(et[96:128, :, 1:ND],
                                         m[96:128, XW - ND:XW, 1:ND],
                                         EC[96:128, :, 1:ND], op=AL.mult),
                 waits=((s_t, T_MM[1]),))
            assert e.n == G_ET[1], e.n
            # den fixups (ered reduced on DVE; den halves written by DVE)
            e.op(lambda: g.tensor_tensor(den[0:32, 0:ND], den[0:32, 0:ND],
                                         ered[0:32], op=AL.subtract),
                 waits=((s_v, V_ERED[0]),))
            assert e.n == G_DEN[0], e.n
            e.op(lambda: g.tensor_tensor(den[96:128, XW - ND:XW],
                                         den[96:128, XW - ND:XW],
                                         ered[96:128], op=AL.subtract),
                 waits=((s_v, V_ERED[1]),))
            assert e.n == G_DEN[1], e.n

        @block.scalar
        def _(act: bass.BassEngine):
            e = Eng(act, s_t)
            act.dma_start(out=A, in_=aa_d).then_inc(s_a, 16)
            # 1: warm the exp/ln table set while DMAs run
            e.op(lambda: act.activation(WRM, CB0, AF.Exp, bias=CB0),
                 waits=((s_g, G_CB),))
            # 2,3: rden = 1/(6a+9) = Exp(-Ln(6a+9))
            e.op(lambda: act.activation(lden, A, AF.Ln, bias=CB9, scale=6.0),
                 waits=((s_a, 16),))
            e.op(lambda: act.activation(rden, lden, AF.Exp,
                                        bias=CB0, scale=-1.0), after=2)
            assert e.n == T_RDEN, e.n
            # 4-9: E1 = Exp(c4*q + c0); m = Exp(-E1) in a d>=1 slab and a
            # d=0 slab with bias -ln2 (m_0 comes out pre-halved)
            for k in range(2):
                hs = slice(k * HW_, (k + 1) * HW_)
                e.op(lambda k=k: act.activation(h(E1, k), h(q, k), AF.Exp,
                                                bias=CG0, scale=float(C4)),
                     waits=((s_v, V_POLY[k]),))
                e.op(lambda hs=hs: act.activation(
                    m[:, hs, 1:ND], E1[:, hs, 1:ND], AF.Exp,
                    bias=CB0, scale=-1.0), after=e.n)
                assert e.n == T_MM[k], e.n
                e.op(lambda hs=hs: act.activation(
                    m[:, hs, 0:1], E1[:, hs, 0:1], AF.Exp,
                    bias=CL2, scale=-1.0), after=e.n)
                assert e.n == T_M[k], e.n
            # 10-13: rdn = 1/den per half (den fixups land on GpSimd)
            for k in range(2):
                e.op(lambda k=k: act.activation(h(lden2, k), h(den, k),
                                                AF.Ln, bias=CB0),
                     waits=((s_g, G_DEN[k]),))
                e.op(lambda k=k: act.activation(h(rdn, k), h(lden2, k),
                                                AF.Exp, bias=CB0, scale=-1.0),
                     after=e.n)
                assert e.n == T_RDN[k], e.n
            act.wait_ge(s_v, V_OUT[1])
            act.dma_start(out=out_d[:, HW_:XW],
                          in_=O[:, HW_:XW]).then_inc(s_fin, 16)

        @block.vector
        def _(v: bass.BassEngine):
            e = Eng(v, s_v)
            dsq_b = DSQ.unsqueeze(1).broadcast_to([128, XW, ND])
            asq_b = asq.unsqueeze(2).broadcast_to([128, XW, ND])
            rden_b = rden.unsqueeze(2).broadcast_to([128, XW, ND])
            # 1: asq (needs aa); 2-3: argA half 0 + tap sums in the rden gap
            e.op(lambda: v.tensor_tensor(asq, A, A, op=AL.mult),
                 waits=((s_a, 16),))
            e.op(lambda: v.tensor_tensor(h(arg, 0), h(dsq_b, 0), h(asq_b, 0),
                                         op=AL.subtract),
                 after=1, waits=((s_g, G_DSQ),))
            e.op(lambda: v.tensor_tensor(xs, xm_v, xp_v, op=AL.add),
                 waits=((s_x, 16),))
            # 4-12: argB + Horner per half, argA half 1 in between
            e.op(lambda: v.tensor_tensor(h(arg, 0), h(arg, 0), h(rden_b, 0),
                                         op=AL.mult),
                 after=2, waits=((s_t, T_RDEN),))
            for k in range(2):
                if k == 1:
                    e.op(lambda: v.tensor_tensor(h(arg, 1), h(dsq_b, 1),
                                                 h(asq_b, 1),
                                                 op=AL.subtract), after=1)
                    e.op(lambda: v.tensor_tensor(h(arg, 1), h(arg, 1),
                                                 h(rden_b, 1), op=AL.mult),
                         after=e.n)
                e.op(lambda k=k: v.scalar_tensor_tensor(
                    h(q, k), h(arg, k), float(U3), h(arg, k),
                    op0=AL.add, op1=AL.mult), after=e.n)
                e.op(lambda k=k: v.scalar_tensor_tensor(
                    h(q, k), h(q, k), float(U2), h(arg, k),
                    op0=AL.add, op1=AL.mult), after=e.n)
                e.op(lambda k=k: v.scalar_tensor_tensor(
                    h(q, k), h(q, k), float(U1), h(arg, k),
                    op0=AL.add, op1=AL.mult), after=e.n)
                assert e.n == V_POLY[k], e.n
            # 13-24: per-half tails, interleaved to hide the rdn waits
            e.op(lambda: v.tensor_reduce(h(S, 0), h(m, 0),
                                         axis=mybir.AxisListType.X,
                                         op=AL.add),
                 waits=((s_t, T_M[0]),))
            e.op(lambda: v.tensor_scalar(h(den, 0), h(S, 0), 2.0, 0.0,
                                         op0=AL.mult, op1=AL.add),
                 after=e.n)
            e.op(lambda: v.tensor_reduce(ered[0:32], et[0:32, :, 1:ND],
                                         axis=mybir.AxisListType.X,
                                         op=AL.add),
                 waits=((s_g, G_ET[0]),))
            assert e.n == V_ERED[0], e.n
            e.op(lambda: v.tensor_tensor(h(mp, 0), h(m, 0), h(xs, 0),
                                         op=AL.mult), after=3)
            e.op(lambda: v.tensor_reduce(h(num, 0), h(mp, 0),
                                         axis=mybir.AxisListType.X,
                                         op=AL.add), after=e.n)
            e.op(lambda: v.tensor_reduce(h(S, 1), h(m, 1),
                                         axis=mybir.AxisListType.X,
                                         op=AL.add),
                 waits=((s_t, T_M[1]),))
            e.op(lambda: v.tensor_scalar(h(den, 1), h(S, 1), 2.0, 0.0,
                                         op0=AL.mult, op1=AL.add),
                 after=e.n)
            e.op(lambda: v.tensor_reduce(ered[96:128], et[96:128, :, 1:ND],
                                         axis=mybir.AxisListType.X,
                                         op=AL.add),
                 waits=((s_g, G_ET[1]),))
            assert e.n == V_ERED[1], e.n
            e.op(lambda: v.tensor_tensor(h(O, 0), h(num, 0), h(rdn, 0),
                                         op=AL.mult),
                 after=17, waits=((s_t, T_RDN[0]),))
            assert e.n == V_OUT[0], e.n
            e.op(lambda: v.tensor_tensor(h(mp, 1), h(m, 1), h(xs, 1),
                                         op=AL.mult), after=3)
            e.op(lambda: v.tensor_reduce(h(num, 1), h(mp, 1),
                                         axis=mybir.AxisListType.X,
                                         op=AL.add), after=e.n)
            e.op(lambda: v.tensor_tensor(h(O, 1), h(num, 1), h(rdn, 1),
                                         op=AL.mult),
                 after=e.n, waits=((s_t, T_RDN[1]),))
            assert e.n == V_OUT[1], e.n

    return nc


_NC_CACHE = None


def _get_nc():
    global _NC_CACHE
    if _NC_CACHE is None:
        _NC_CACHE = build_bass()
    return _NC_CACHE


def _ec_host():
    k = np.arange(ND)[:, None]
    d = np.arange(ND)[None, :]
    ec = np.zeros((128, ND, ND), np.float32)
    ec[0:16] = (d > k).astype(np.float32)
    ec[112:128] = ((d + k) > W).astype(np.float32)
    return ec.reshape(128, ND * ND)


def make_in_maps(x, aa):
    import ml_dtypes
    x = np.asarray(x, dtype=np.float32)
    aa = np.asarray(aa, dtype=np.float32)
    ec = _ec_host()
    in_maps = []
    for b in range(NC_COUNT):
        xp = np.pad(x[b], ((0, 0), (HALO, HALO)))   # (16, 1036)
        xe = np.empty((128, PITCH), np.float32)
        xh = np.stack([xp[:, c * XW:c * XW + XROW] for c in range(NCH)])
        xe[:, 0:XROW] = xh.reshape(128, XROW)
        xe[:, XROW:] = ec
        ah = np.stack([aa[b][:, c * XW:(c + 1) * XW] for c in range(NCH)])
        in_maps.append({"xe": xe.astype(ml_dtypes.bfloat16),
                        "aa": ah.reshape(128, XW).copy()})
    return in_maps


def gather_out(o):
    return np.asarray(o).reshape(NCH, L, XW).transpose(1, 0, 2).reshape(L, F)


def kernel(x, aa):
    nc = _get_nc()
    res = run_bass_kernel_spmd(nc, make_in_maps(x, aa),
                               core_ids=list(range(NC_COUNT)))
    return np.stack([gather_out(res.results[b]["out"])
                     for b in range(NC_COUNT)], axis=0)


# revision 15
# speedup vs baseline: 1.2042x; 1.2042x over previous
"""BumpX pooling kernel for Trainium2 (8 NeuronCores, data-parallel over batch).

Math (per batch b, row l, position i, a = aa[b,l,i], d = |j - i|):
    arg_d = (d^2 - a^2) / (6a + 9)
    m_d   = 1 - gg(arg_d)        (the bump mask; underflows for d >= 7)
    out_i = sum_d m_d (x[i-d] + x[i+d]) / sum_d m_d n_valid(i, d)

Approximations (rel tolerance 2e-2; this lands ~6.5e-3):
  - m(t) = exp(-exp(g(t))), g fitted by a degree-3 polynomial over
    t in [-0.08, 4.01] (the full arg range for d <= 6); d=7 dropped.
  - mask/tap stacks held in bf16 (2x DVE rate on packed tensor_tensor);
    reductions accumulate in fp32.

Per-element chain: arg (2 DVE ops; 1/(6a+9) comes from ACT's
Ln(6a+9)/Exp with the affine folded into scale/bias) -> monic Horner q
(2 fused DVE ops) -> E1 = Exp(c3*q + c0) (ACT) -> m = Exp(-E1) (ACT).
The d=0 mask is halved in place so the symmetric-tap stack
xs_d = x[i-d] + x[i+d] (ONE DVE op via a +/-1-stride view pair,
xs_0 = 2x) gives num = reduce(m*xs) and den = 2*reduce(m). Row-edge
taps are removed with masked products on GpSimd + small reductions.

The stacks are processed in two position halves software-pipelined
across DVE and ACT; each half's output columns are DMAed as soon as its
numerator/denominator finish (SP stores half 0, ACT half 1). No engine
waits for output-DMA completion: the NEFF epilogue's per-queue drain
waits already guarantee the stores land before completion is signaled.

Layout per core (core = batch): partition p = c*16 + l (c = chunk of 128
positions, l = row); stacks are (128, 128, 7) with d innermost so the
d-reduction is a contiguous X-reduce. Inputs arrive as two 2D DMAs:
bf16 [x-halo(140) | edge-masks(49)] per partition, and fp32 aa (128x128).
"""

import numpy as np

import concourse.bass as bass
import concourse.mybir as mybir
from concourse.bass_utils import run_bass_kernel_spmd

F32 = mybir.dt.float32
BF16 = mybir.dt.bfloat16
L, F = 16, 1024
NC_COUNT = 8
W = 6          # max diagonal distance kept
ND = W + 1     # stack depth (d = 0..6)
HALO = W
XW = 128       # positions per chunk
HW_ = XW // 2  # position half width
NCH = F // XW  # 8 chunks
XROW = XW + 2 * HALO           # 140
PITCH = XROW + ND * ND         # 189: [x-halo | EC masks]


def _fit_g():
    """Degree-3 weighted fit of g(t) = ln(-ln(m(t))) over the arg range."""
    t = np.linspace(-0.08, 4.01, 20001)
    sp = np.log1p(np.exp(-np.abs(t))) + np.maximum(t, 0)
    spr = np.log1p(np.exp(-np.abs(1 - t))) + np.maximum(1 - t, 0)
    mm = np.exp(-1.0 / np.clip(spr, 1e-6, None))
    mm = mm / (mm + np.exp(-1.0 / np.clip(sp, 1e-6, None)))
    g = np.log(-np.log(mm))
    w = np.abs(mm * np.log(mm)) + 1e-6
    return np.polyfit(t, g, 3, w=w)


C3, C2, C1, C0 = (float(v) for v in _fit_g())
U2, U1 = C2 / C3, C1 / C3


class _FastBass(bass.Bass):
    """Skip the constructor's all-engine barrier (~3us): we never read the
    framework's const APs (all ACT biases are explicit tiles)."""

    def all_engine_barrier(self, *, sem_only: bool = False):
        if not getattr(self, "_init_barrier_skipped", False):
            self._init_barrier_skipped = True
            return
        return super().all_engine_barrier(sem_only=sem_only)


def build_bass():
    nc = _FastBass("TRN2", debug=False)

    xe_d = nc.dram_tensor("xe", [128, PITCH], BF16, kind="ExternalInput").ap()
    aa_d = nc.dram_tensor("aa", [128, XW], F32, kind="ExternalInput").ap()
    out_d = nc.dram_tensor("out", [128, XW], F32, kind="ExternalOutput").ap()

    def sb(name, shape, dt=F32):
        return nc.alloc_sbuf_tensor(name, shape, dt).ap()

    XE = sb("XE", [128, PITCH], BF16)  # [x-halo(140) | EC(7,7)(49)]
    A = sb("A", [128, XW])
    DSQ = sb("DSQ", [128, ND])
    CB0 = sb("CB0", [128, 1])          # 0.0   (ACT bias)
    CB9 = sb("CB9", [128, 1])          # 9.0   (ACT bias: Ln(6a+9))
    CG0 = sb("CG0", [128, 1])          # C0    (ACT bias for E1)
    WRM = sb("WRM", [128, 1])
    lden = sb("lden", [128, XW])
    rden = sb("rden", [128, XW])
    asq = sb("asq", [128, XW])
    arg = sb("arg", [128, XW, ND], BF16)
    q = sb("q", [128, XW, ND], BF16)
    E1 = sb("E1", [128, XW, ND])
    m = sb("m", [128, XW, ND], BF16)
    xs = sb("xs", [128, XW, ND], BF16)
    mp = sb("mp", [128, XW, ND], BF16)
    S = sb("S", [128, XW])
    den = sb("den", [128, XW])
    lden2 = sb("lden2", [128, XW])
    rdn = sb("rdn", [128, XW])
    num = sb("num", [128, XW])
    O = sb("O", [128, XW])
    et = sb("et", [128, ND, ND], BF16)  # edge products ([0:32] / [96:128])
    ered = sb("ered", [128, ND])

    # EC view: XE[:, 140:189] seen as (128, 7, 7) [k, d]
    EC = bass.AP(tensor=XE.tensor, offset=XROW,
                 ap=[[PITCH, 128], [ND, ND], [1, ND]])
    # xs operand views: elem (p, i, d) -> XE[p, HALO + i -/+ d]
    xm_v = bass.AP(tensor=XE.tensor, offset=HALO,
                   ap=[[PITCH, 128], [1, XW], [-1, ND]])
    xp_v = bass.AP(tensor=XE.tensor, offset=HALO,
                   ap=[[PITCH, 128], [1, XW], [1, ND]])

    AL = mybir.AluOpType
    AF = mybir.ActivationFunctionType

    def h(t, k):
        """Column-half slice of a (128, XW, ...) AP."""
        return t[:, k * HW_:(k + 1) * HW_]

    class Eng:
        """Engine op wrapper with minimal-dependency waits.

        Engines issue and COMPLETE instructions in order, but a later
        instruction's reads can start before an earlier one's writes land,
        so every data hazard needs a semaphore wait. Each op incs the
        engine's chain sem on completion; `after=k` waits for the first k
        chained ops. Redundant waits (value already awaited) are skipped."""

        def __init__(self, eng, sem):
            self.eng, self.sem, self.n = eng, sem, 0
            self.waited = {}

        def wait(self, sem, val):
            key = id(sem)
            if self.waited.get(key, -1) < val:
                self.eng.wait_ge(sem, val)
                self.waited[key] = val

        def op(self, make_inst, after=0, waits=()):
            for sem, val in waits:
                self.wait(sem, val)
            if after:
                self.wait(self.sem, after)
            inst = make_inst()
            inst.then_inc(self.sem, 1)
            self.n += 1
            assert self.n >= after
            return inst

    with (
        nc.Block(no_gpsimd_drain=True) as block,
        nc.semaphore("s_a") as s_a,
        nc.semaphore("s_x") as s_x,
        nc.semaphore("s_fin") as s_fin,
        nc.semaphore("s_v") as s_v,      # DVE chain
        nc.semaphore("s_t") as s_t,      # ACT chain
        nc.semaphore("s_g") as s_g,      # GPSIMD chain
    ):
        # chain-count milestones
        G_CB = 3
        G_DSQ = 10
        G_ET = (11, 13)
        G_DEN = (12, 14)
        V_POLY = (6, 10)
        V_DENRED = (14, 21)
        V_OUT = (17, 24)
        T_RDEN = 3
        T_M = (5, 7)
        T_RDN = (9, 11)

        @block.sync
        def _(sync: bass.BassEngine):
            sync.dma_start(out=XE, in_=xe_d).then_inc(s_x, 16)
            sync.wait_ge(s_v, V_OUT[0])
            sync.dma_start(out=out_d[:, 0:HW_],
                           in_=O[:, 0:HW_]).then_inc(s_fin, 16)

        @block.gpsimd
        def _(g: bass.BassEngine):
            e = Eng(g, s_g)
            e.op(lambda: g.memset(CB0, 0.0))
            e.op(lambda: g.memset(CB9, 9.0))
            e.op(lambda: g.memset(CG0, float(C0)))
            assert e.n == G_CB, e.n
            for d in range(ND):
                e.op(lambda d=d: g.memset(DSQ[:, d:d + 1], float(d * d)))
            assert e.n == G_DSQ, e.n
            # edge products (d >= 1) + den fixups, interleaved so the half-0
            # fixup is not queued behind the half-1 product
            e.op(lambda: g.tensor_tensor(et[0:32, :, 1:ND],
                                         m[0:32, 0:ND, 1:ND],
                                         EC[0:32, :, 1:ND], op=AL.mult),
                 waits=((s_t, T_M[0]), (s_x, 16)))
            assert e.n == G_ET[0], e.n
            e.op(lambda: g.tensor_tensor(den[0:32, 0:ND], den[0:32, 0:ND],
                                         ered[0:32], op=AL.subtract),
                 waits=((s_v, V_DENRED[0]),))
            assert e.n == G_DEN[0], e.n
            e.op(lambda: g.tensor_tensor(et[96:128, :, 1:ND],
                                         m[96:128, XW - ND:XW, 1:ND],
                                         EC[96:128, :, 1:ND], op=AL.mult),
                 waits=((s_t, T_M[1]),))
            assert e.n == G_ET[1], e.n
            e.op(lambda: g.tensor_tensor(den[96:128, XW - ND:XW],
                                         den[96:128, XW - ND:XW],
                                         ered[96:128], op=AL.subtract),
                 waits=((s_v, V_DENRED[1]),))
            assert e.n == G_DEN[1], e.n

        @block.scalar
        def _(act: bass.BassEngine):
            e = Eng(act, s_t)
            act.dma_start(out=A, in_=aa_d).then_inc(s_a, 16)
            # 1: warm the exp/ln table set while DMAs run
            e.op(lambda: act.activation(WRM, CB0, AF.Exp, bias=CB0),
                 waits=((s_g, G_CB),))
            # 2,3: rden = 1/(6a+9) = Exp(-Ln(6a+9))
            e.op(lambda: act.activation(lden, A, AF.Ln, bias=CB9, scale=6.0),
                 waits=((s_a, 16),))
            e.op(lambda: act.activation(rden, lden, AF.Exp,
                                        bias=CB0, scale=-1.0), after=2)
            assert e.n == T_RDEN, e.n
            # 4-7: E1 = Exp(c3*q + c0); m = Exp(-E1), per half
            for k in range(2):
                e.op(lambda k=k: act.activation(h(E1, k), h(q, k), AF.Exp,
                                                bias=CG0, scale=float(C3)),
                     waits=((s_v, V_POLY[k]),))
                e.op(lambda k=k: act.activation(h(m, k), h(E1, k), AF.Exp,
                                                bias=CB0, scale=-1.0),
                     after=e.n)
                assert e.n == T_M[k], e.n
            # 8-11: rdn = 1/den per half (den fixups land on GpSimd)
            for k in range(2):
                e.op(lambda k=k: act.activation(h(lden2, k), h(den, k),
                                                AF.Ln, bias=CB0),
                     waits=((s_g, G_DEN[k]),))
                e.op(lambda k=k: act.activation(h(rdn, k), h(lden2, k),
                                                AF.Exp, bias=CB0, scale=-1.0),
                     after=e.n)
                assert e.n == T_RDN[k], e.n
            act.wait_ge(s_v, V_OUT[1])
            act.dma_start(out=out_d[:, HW_:XW],
                          in_=O[:, HW_:XW]).then_inc(s_fin, 16)

        @block.vector
        def _(v: bass.BassEngine):
            e = Eng(v, s_v)
            dsq_b = DSQ.unsqueeze(1).broadcast_to([128, XW, ND])
            asq_b = asq.unsqueeze(2).broadcast_to([128, XW, ND])
            rden_b = rden.unsqueeze(2).broadcast_to([128, XW, ND])
            # 1: asq (needs aa); 2: argA half 0; 3: tap sums in the rden gap
            e.op(lambda: v.tensor_tensor(asq, A, A, op=AL.mult),
                 waits=((s_a, 16),))
            e.op(lambda: v.tensor_tensor(h(arg, 0), h(dsq_b, 0), h(asq_b, 0),
                                         op=AL.subtract),
                 after=1, waits=((s_g, G_DSQ),))
            e.op(lambda: v.tensor_tensor(xs, xm_v, xp_v, op=AL.add),
                 waits=((s_x, 16),))
            # 4-10: argB + monic Horner per half, argA half 1 in between
            e.op(lambda: v.tensor_tensor(h(arg, 0), h(arg, 0), h(rden_b, 0),
                                         op=AL.mult),
                 after=2, waits=((s_t, T_RDEN),))
            for k in range(2):
                if k == 1:
                    e.op(lambda: v.tensor_tensor(h(arg, 1), h(dsq_b, 1),
                                                 h(asq_b, 1),
                                                 op=AL.subtract), after=1)
                    e.op(lambda: v.tensor_tensor(h(arg, 1), h(arg, 1),
                                                 h(rden_b, 1), op=AL.mult),
                         after=e.n)
                e.op(lambda k=k: v.scalar_tensor_tensor(
                    h(q, k), h(arg, k), float(U2), h(arg, k),
                    op0=AL.add, op1=AL.mult), after=e.n)
                e.op(lambda k=k: v.scalar_tensor_tensor(
                    h(q, k), h(q, k), float(U1), h(arg, k),
                    op0=AL.add, op1=AL.mult), after=e.n)
                assert e.n == V_POLY[k], e.n
            # 11-24: per-half tails
            for k in range(2):
                mh = m[:, k * HW_:(k + 1) * HW_, 0:1]
                e.op(lambda mh=mh: v.tensor_scalar(mh, mh, 0.5, 0.0,
                                                   op0=AL.mult, op1=AL.add),
                     waits=((s_t, T_M[k]),))
                m0h = e.n
                e.op(lambda k=k: v.tensor_reduce(h(S, k), h(m, k),
                                                 axis=mybir.AxisListType.X,
                                                 op=AL.add), after=m0h)
                if k == 0:
                    e.op(lambda: v.tensor_reduce(ered[0:32],
                                                 et[0:32, :, 1:ND],
                                                 axis=mybir.AxisListType.X,
                                                 op=AL.add),
                         waits=((s_g, G_ET[0]),))
                else:
                    e.op(lambda: v.tensor_reduce(ered[96:128],
                                                 et[96:128, :, 1:ND],
                                                 axis=mybir.AxisListType.X,
                                                 op=AL.add),
                         waits=((s_g, G_ET[1]),))
                e.op(lambda k=k: v.tensor_scalar(h(den, k), h(S, k), 2.0, 0.0,
                                                 op0=AL.mult, op1=AL.add),
                     after=m0h + 1)
                assert e.n == V_DENRED[k], e.n
                e.op(lambda k=k: v.tensor_tensor(h(mp, k), h(m, k), h(xs, k),
                                                 op=AL.mult), after=m0h)
                e.op(lambda k=k: v.tensor_reduce(h(num, k), h(mp, k),
                                                 axis=mybir.AxisListType.X,
                                                 op=AL.add), after=e.n)
                e.op(lambda k=k: v.tensor_tensor(h(O, k), h(num, k),
                                                 h(rdn, k), op=AL.mult),
                     after=e.n, waits=((s_t, T_RDN[k]),))
                assert e.n == V_OUT[k], e.n

    return nc


_NC_CACHE = None


def _get_nc():
    global _NC_CACHE
    if _NC_CACHE is None:
        _NC_CACHE = build_bass()
    return _NC_CACHE


def _ec_host():
    k = np.arange(ND)[:, None]
    d = np.arange(ND)[None, :]
    ec = np.zeros((128, ND, ND), np.float32)
    ec[0:16] = (d > k).astype(np.float32)
    ec[112:128] = ((d + k) > W).astype(np.float32)
    return ec.reshape(128, ND * ND)


def make_in_maps(x, aa):
    import ml_dtypes
    x = np.asarray(x, dtype=np.float32)
    aa = np.asarray(aa, dtype=np.float32)
    ec = _ec_host()
    in_maps = []
    for b in range(NC_COUNT):
        xp = np.pad(x[b], ((0, 0), (HALO, HALO)))   # (16, 1036)
        xe = np.empty((128, PITCH), np.float32)
        xh = np.stack([xp[:, c * XW:c * XW + XROW] for c in range(NCH)])
        xe[:, 0:XROW] = xh.reshape(128, XROW)
        xe[:, XROW:] = ec
        ah = np.stack([aa[b][:, c * XW:(c + 1) * XW] for c in range(NCH)])
        in_maps.append({"xe": xe.astype(ml_dtypes.bfloat16),
                        "aa": ah.reshape(128, XW).copy()})
    return in_maps


def gather_out(o):
    return np.asarray(o).reshape(NCH, L, XW).transpose(1, 0, 2).reshape(L, F)


def kernel(x, aa):
    nc = _get_nc()
    res = run_bass_kernel_spmd(nc, make_in_maps(x, aa),
                               core_ids=list(range(NC_COUNT)))
    return np.stack([gather_out(res.results[b]["out"])
                     for b in range(NC_COUNT)], axis=0)
